# revision 1
# baseline (speedup 1.0000x reference)
"""Deformable spatial attention layer — Trainium2 Bass kernel.

Full inputs in, full outputs out. Sharding: 8 cores = 2 batches x 4 horizontal
bands of 32 image rows (128x128 image, 16384 queries, 8 heads x 4 points,
head_dim 32).

Sampling locations are query_pixel + off where off = q@W_off + b_off has a
small data-dependent spread around integer directional biases, so bilinear
sampling becomes a small set of integer-shift multiply-accumulates ("shift
enumeration"): per head and integer shift (oy, ox) in the data-derived support
window, samp += coeff(q) * img[q + (oy,ox)], with coeff a product of bilinear
hat functions and softmaxed attention weights.  Supports are computed host-side
from the actual offsets (with margin) and baked in as constants.

Layouts: x (image column) on partitions everywhere.  The per-head value image
is bf16 [x, head, d, y] (y innermost) so y-shifts are cheap aligned free-dim
offsets that keep the DVE 2x bf16 mode.  x-shifts need cross-partition moves,
which engines cannot do, so per-(head, ox) shifted copies are built with
SBUF->SBUF DMAs (DMA addresses partitions freely); zero-filled gaps implement
the zero-padding boundary condition.  Accumulation into samp uses one
tensor_tensor add per (head, ox) whose output AP revisits the same samp region
per cell (sequential in-stream read-modify-write, verified on HW).
"""

import os
import sys

import numpy as np
import ml_dtypes

for _p in ("/opt/trn_rl_repo", "/root/.axon_site/_ro/trn_rl_repo"):
    if os.path.isdir(_p) and _p not in sys.path:
        sys.path.insert(0, _p)

import concourse.bass as bass  # noqa: E402
import concourse.mybir as mybir  # noqa: E402
from concourse.bacc import Bacc  # noqa: E402
from concourse.tile import TileContext  # noqa: E402
from concourse.bass_utils import run_bass_kernel_spmd  # noqa: E402

F32 = mybir.dt.float32
F32R = mybir.dt.float32r
BF16 = mybir.dt.bfloat16
OP = mybir.AluOpType

NH, NP, D = 8, 4, 32
H = W = 128
NQ = H * W
CIN = COUT = 256
NB = 4          # bands per batch
BAND = H // NB  # 32 rows per band
EPS = 0.01

# (head, ox) groups whose shift-accumulate runs on gpsimd ("*" = all ox)
GPSIMD_GROUPS = {0: "*", 4: "*"}


def _host_meta(query, W_off, b_off):
    """Data-derived support windows; cells grouped per (head, ox)."""
    q2 = query.reshape(-1, CIN).astype(np.float32)
    off = (q2 @ W_off + b_off).reshape(-1, NH, NP, 2)
    offx = off[..., 0]
    offy = off[..., 1]

    basex = np.floor(offx.min(0) - EPS).astype(np.int64)      # [NH, NP]
    basey = np.floor(offy.min(0) - EPS).astype(np.int64)
    wx = (np.floor(offx.max(0) + EPS) + 2 - basex).astype(np.int64)
    wy = (np.floor(offy.max(0) + EPS) + 2 - basey).astype(np.int64)
    maxw = int(max(wx.max(), wy.max()))

    heads = []
    halo_t = 0
    halo_b = 0
    for h in range(NH):
        cells = {}
        for p in range(NP):
            for jy in range(int(wy[h, p])):
                for jx in range(int(wx[h, p])):
                    oy = int(basey[h, p]) + jy
                    ox = int(basex[h, p]) + jx
                    cells.setdefault((oy, ox), []).append((p, jy, jx))
        groups = {}
        for (oy, ox), ct in sorted(cells.items(), key=lambda c: (c[0][1], c[0][0])):
            groups.setdefault(ox, []).append((oy, ct))
        heads.append({"groups": sorted(groups.items())})
        halo_t = max(halo_t, -min(oy for (oy, _) in cells))
        halo_b = max(halo_b, max(oy for (oy, _) in cells))

    BH = BAND + halo_t + halo_b
    BH += BH % 2  # keep y-length even so d-slices stay 4B aligned in bf16
    return {
        "basex": basex, "basey": basey, "maxw": maxw, "heads": heads,
        "halo_t": halo_t, "halo_b": halo_b, "BH": BH,
    }


def _build_program(meta):
    BH = meta["BH"]
    maxw = meta["maxw"]
    nc = Bacc()

    def f32r(ap):
        return ap.bitcast(F32R)

    # ---------------- DRAM I/O ----------------
    d_val = nc.dram_tensor("valpad", [BH * W, CIN], BF16, kind="ExternalInput")
    d_qry = nc.dram_tensor("qband", [BAND * W, CIN], F32, kind="ExternalInput")
    d_wv = nc.dram_tensor("wval", [CIN, COUT], BF16, kind="ExternalInput")
    d_woa = nc.dram_tensor("woa", [CIN, 96], F32, kind="ExternalInput")
    d_wo = nc.dram_tensor("wout", [COUT, COUT], BF16, kind="ExternalInput")
    d_id = nc.dram_tensor("ident", [128, 128], F32, kind="ExternalInput")
    d_idb = nc.dram_tensor("identb", [128, 128], BF16, kind="ExternalInput")
    d_cb = nc.dram_tensor("cb", [128, 64], F32, kind="ExternalInput")
    d_ones = nc.dram_tensor("onesrow", [1, 128], F32, kind="ExternalInput")
    d_boa = nc.dram_tensor("boa", [1, 96], F32, kind="ExternalInput")
    d_bv = nc.dram_tensor("bvrow", [1, COUT], F32, kind="ExternalInput")
    d_bo = nc.dram_tensor("borow", [1, COUT], F32, kind="ExternalInput")
    d_zg = nc.dram_tensor("zgap", [16, BH * D], BF16, kind="ExternalInput")
    d_out = nc.dram_tensor("out", [BAND * W, COUT], F32, kind="ExternalOutput")

    with TileContext(nc) as tc:
        with (
            tc.tile_pool(name="const", bufs=1) as Pc,
            tc.tile_pool(name="img", bufs=1) as Pimg,
            tc.tile_pool(name="samp", bufs=1) as Psamp,
            tc.tile_pool(name="uc", bufs=1) as Puc,
            tc.tile_pool(name="prods", bufs=1) as Pprod,
            tc.tile_pool(name="psA", bufs=2, space="PSUM") as PSa,
            tc.tile_pool(name="psB", bufs=4, space="PSUM") as PSb,
        ):
            # ---- constants ----
            t_wv = Pc.tile([128, 2, COUT], BF16)
            t_woa = Pc.tile([128, 2, 96], F32)
            t_wo = Pc.tile([128, 2, COUT], BF16)
            t_id = Pc.tile([128, 128], F32)
            t_idb = Pc.tile([128, 128], BF16)
            t_cb = Pc.tile([128, 64], F32)
            t_ones = Pc.tile([1, 128], F32)
            t_boa = Pc.tile([1, 96], F32)
            t_bv = Pc.tile([1, COUT], F32)
            t_bo = Pc.tile([1, COUT], F32)
            nc.sync.dma_start(t_wv[:], d_wv[:].rearrange("(k p) c -> p k c", p=128))
            nc.sync.dma_start(t_woa[:], d_woa[:].rearrange("(k p) c -> p k c", p=128))
            nc.sync.dma_start(t_wo[:], d_wo[:].rearrange("(k p) c -> p k c", p=128))
            nc.sync.dma_start(t_id[:], d_id[:])
            nc.sync.dma_start(t_idb[:], d_idb[:])
            nc.sync.dma_start(t_cb[:], d_cb[:])
            nc.sync.dma_start(t_ones[:], d_ones[:])
            nc.sync.dma_start(t_boa[:], d_boa[:])
            nc.sync.dma_start(t_bv[:], d_bv[:])
            nc.sync.dma_start(t_bo[:], d_bo[:])

            # ---- persistent tiles ----
            t_img = Pimg.tile([128, NH, D, BH], BF16)       # [x, h, d, iy]
            t_samp = Psamp.tile([128, NH, D, BAND], BF16)   # [x, h, d, y]
            n_multi = []
            for hd in meta["heads"]:
                n_multi.append(sum(1 for (_, cl) in hd["groups"]
                                   for (_, ct) in cl if len(ct) > 1))
            t_uc = [Puc.tile([128, max(1, m), BAND], BF16, tag=f"uc{h}",
                             name=f"uc{h}")
                    for h, m in enumerate(n_multi)]
            t_pr = [[Pprod.tile([128, 32, BAND], BF16, tag=f"pr{jy}_{jx}",
                                name=f"pr{jy}_{jx}")
                     for jx in range(maxw)] for jy in range(maxw)]

            # value + query(G) loads issued up front so DMA overlaps C/D
            _q2_cm = tc.tile_pool(name="q2", bufs=1)
            Pq2 = _q2_cm.__enter__()
            t_q2 = Pq2.tile([128, BAND, CIN], F32)
            nc.sync.dma_start(
                t_q2[:], d_qry[:].rearrange("(y x) c -> x y c", x=128))
            _v_cm = tc.tile_pool(name="vt", bufs=1)
            Pv = _v_cm.__enter__()
            t_v = Pv.tile([128, BH, CIN], BF16)
            vch = (BH + 2) // 3
            vv = d_val[:].rearrange("(y x) c -> x y c", x=128)
            for c0 in range(0, BH, vch):
                c1 = min(BH, c0 + vch)
                nc.sync.dma_start(t_v[:, c0:c1, :], vv[:, c0:c1, :])

            # ================= C: query transposes + off/attn proj ==========
            _off_cm = tc.tile_pool(name="off", bufs=1)
            Poff = _off_cm.__enter__()
            t_off = Poff.tile([128, BAND, 96], F32)        # [x, y, col]
            with tc.tile_pool(name="rot", bufs=4) as Prot:
                t_q = t_q2
                for yc in range(BAND):
                    qT = []
                    for k in range(2):
                        pT = PSa.tile([128, 128], F32, tag="trT", name="pT")
                        nc.tensor.transpose(
                            pT[:], t_q[:, yc, 128 * k:128 * (k + 1)], t_id[:])
                        sT = Prot.tile([128, 128], F32, tag="qT", name="sT")
                        nc.scalar.copy(sT[:], pT[:])
                        qT.append(sT)
                    pO = PSb.tile([128, COUT], F32, tag="proj", name="pO")
                    nc.tensor.matmul(pO[:, 0:96], qT[0][:], t_woa[:, 0, :],
                                     start=True, stop=False)
                    nc.tensor.matmul(pO[:, 0:96], qT[1][:], t_woa[:, 1, :],
                                     start=False, stop=False)
                    nc.tensor.matmul(pO[:, 0:96], t_ones[:], t_boa[:],
                                     start=False, stop=True)
                    nc.scalar.copy(t_off[:, yc, :], pO[:, 0:96])

            # ================= D: softmax + taps + products =================
            offv = t_off[:]
            with tc.tile_pool(name="soft", bufs=1) as Ps:
                t_awn = Ps.tile([128, NH, NP, BAND], F32)
                with tc.tile_pool(name="soft2", bufs=1) as Ps2:
                    t_exp = Ps2.tile([128, NH, NP, BAND], F32)
                    t_sum = Ps2.tile([128, NH, BAND], F32)
                    t_rcp = Ps2.tile([128, NH, BAND], F32)
                    logits = offv[:, :, 64:96].rearrange(
                        "x y (h p) -> x h p y", h=NH)
                    nc.scalar.activation(t_exp[:], logits,
                                         mybir.ActivationFunctionType.Exp)
                    nc.vector.tensor_reduce(
                        t_sum[:],
                        t_exp[:].rearrange("x h p y -> x h y p"),
                        mybir.AxisListType.X, OP.add)
                    nc.vector.reciprocal(t_rcp[:], t_sum[:])
                    nc.vector.tensor_tensor(
                        t_awn[:], t_exp[:],
                        t_rcp[:, :, None, :].broadcast_to([128, NH, NP, BAND]),
                        OP.mult)

                with tc.tile_pool(name="txty", bufs=1) as Pt:
                    t_tx = Pt.tile([128, 32, BAND], F32)
                    t_ty = Pt.tile([128, 32, BAND], F32)
                    t_sc = Pt.tile([128, 32, BAND], F32)
                    offxy = offv[:, :, 0:64].rearrange(
                        "x y (h p t) -> x t (h p) y", h=NH, p=NP)
                    cbx = t_cb[:, 0:32, None].broadcast_to([128, 32, BAND])
                    cby = t_cb[:, 32:64, None].broadcast_to([128, 32, BAND])
                    nc.vector.tensor_tensor(t_tx[:], offxy[:, 0], cbx, OP.subtract)
                    nc.vector.tensor_tensor(t_ty[:], offxy[:, 1], cby, OP.subtract)

                    with tc.tile_pool(name="hats", bufs=1) as Ph:
                        t_hx = [Ph.tile([128, 32, BAND], F32, tag=f"hx{j}",
                                        name=f"hx{j}") for j in range(maxw)]
                        t_hy = [Ph.tile([128, 32, BAND], F32, tag=f"hy{j}",
                                        name=f"hy{j}") for j in range(maxw)]

                        def taps(dst, t_src):
                            nc.vector.tensor_scalar(
                                dst[0][:], t_src[:], 1.0, None, OP.min)
                            nc.vector.tensor_scalar(
                                dst[0][:], dst[0][:], -1.0, 1.0, OP.mult, OP.add)
                            for j in range(1, maxw):
                                nc.vector.tensor_scalar(
                                    dst[j][:], t_src[:], -1.0, float(j + 1),
                                    OP.mult, OP.add)          # (j+1)-t
                                if j == 1:
                                    nc.vector.tensor_tensor(
                                        dst[j][:], dst[j][:], t_src[:], OP.min)
                                else:
                                    nc.vector.tensor_scalar(
                                        t_sc[:], t_src[:], -float(j - 1),
                                        None, OP.add)
                                    nc.vector.tensor_tensor(
                                        dst[j][:], dst[j][:], t_sc[:], OP.min)
                                nc.vector.tensor_scalar(
                                    dst[j][:], dst[j][:], 0.0, None, OP.max)

                        taps(t_hx, t_tx)
                        taps(t_hy, t_ty)
                        awf = t_awn[:].rearrange("x h p y -> x (h p) y")
                        for j in range(maxw):
                            nc.vector.tensor_tensor(
                                t_hy[j][:], t_hy[j][:], awf, OP.mult)
                        for jy in range(maxw):
                            for jx in range(maxw):
                                nc.gpsimd.tensor_tensor(
                                    t_pr[jy][jx][:], t_hy[jy][:], t_hx[jx][:],
                                    OP.mult)

            # union-cell coefficients (multi-contributor cells), bf16 adds
            for h, hd in enumerate(meta["heads"]):
                ci = 0
                for (ox, cl) in hd["groups"]:
                    for (oy, ct) in cl:
                        if len(ct) <= 1:
                            continue
                        dst = t_uc[h][:, ci, :]
                        p0, jy0, jx0 = ct[0]
                        p1, jy1, jx1 = ct[1]
                        nc.vector.tensor_tensor(
                            dst, t_pr[jy0][jx0][:, 4 * h + p0, :],
                            t_pr[jy1][jx1][:, 4 * h + p1, :], OP.add)
                        for (p, jy, jx) in ct[2:]:
                            nc.vector.tensor_tensor(
                                dst, dst, t_pr[jy][jx][:, 4 * h + p, :], OP.add)
                        ci += 1
            _off_cm.__exit__(None, None, None)

            # ================= B: value transposes + v-proj (f32r) ==========
            with tc.tile_pool(name="rotv", bufs=4) as Prv:
                for iy in range(BH):
                    vT = []
                    for k in range(2):
                        pT = PSa.tile([128, 128], BF16, tag="trTb", name="pT",
                                      padded_shape=[128, 512])
                        nc.tensor.transpose(
                            pT[:], t_v[:, iy, 128 * k:128 * (k + 1)], t_idb[:])
                        sT = Prv.tile([128, 128], BF16, tag="vT", name="sT")
                        nc.scalar.copy(sT[:], pT[:])
                        vT.append(sT)
                    pV = PSb.tile([128, COUT], F32, tag="proj", name="pV")
                    nc.tensor.matmul(pV[:], vT[0][:], t_wv[:, 0, :],
                                     start=True, stop=False)
                    nc.tensor.matmul(pV[:], vT[1][:], t_wv[:, 1, :],
                                     start=False, stop=False)
                    nc.tensor.matmul(pV[:], t_ones[:], t_bv[:],
                                     start=False, stop=True)
                    nc.scalar.copy(
                        t_img[:, :, :, iy],
                        pV[:].rearrange("x (h d) -> x h d", h=NH))
            _v_cm.__exit__(None, None, None)

            # ================= E+F: shifted copies + shift-accumulate =======
            halo_t = meta["halo_t"]
            with tc.tile_pool(name="imgs", bufs=2) as Psh, \
                 tc.tile_pool(name="imgsg", bufs=2) as Pshg, \
                 tc.tile_pool(name="ptmp", bufs=2) as Ppt, \
                 tc.tile_pool(name="ptmpg", bufs=2) as Pptg:
                for h, hd in enumerate(meta["heads"]):
                    gsel = GPSIMD_GROUPS.get(h)
                    shpool = Pshg if gsel == "*" else Psh
                    kmax = max(len(cl) for (_, cl) in hd["groups"])
                    sh_tiles = {}
                    i = 0
                    for (ox, _) in hd["groups"]:
                        if ox == 0:
                            continue
                        ts_ = shpool.tile([128, D, BH], BF16, tag=f"sh{i}",
                                          name=f"sh{i}")
                        i += 1
                        a = abs(ox)
                        zview = d_zg[0:a, :].rearrange("p (d y) -> p d y", d=D)
                        if ox > 0:
                            nc.sync.dma_start(
                                ts_[0:128 - a, :, :], t_img[a:128, h, :, :])
                            nc.sync.dma_start(ts_[128 - a:128, :, :], zview)
                        else:
                            nc.sync.dma_start(
                                ts_[a:128, :, :], t_img[0:128 - a, h, :, :])
                            nc.sync.dma_start(ts_[0:a, :, :], zview)
                        sh_tiles[ox] = ts_
                    first = True
                    ci = 0
                    for gi, (ox, cl) in enumerate(hd["groups"]):
                        on_gp = gsel == "*" or (gsel is not None and ox in gsel)
                        eng = nc.gpsimd if on_gp else nc.vector
                        ptpool = Pptg if on_gp else Ppt
                        k = len(cl)
                        ptg = ptpool.tile([128, kmax, D, BAND], BF16,
                                          tag="ptg", name="ptg")
                        for c, (oy, ct) in enumerate(cl):
                            iy = halo_t + oy
                            if ox == 0:
                                src = t_img[:, h, :, iy:iy + BAND]
                            else:
                                src = sh_tiles[ox][:, :, iy:iy + BAND]
                            if len(ct) == 1:
                                p, jy, jx = ct[0]
                                cf = t_pr[jy][jx][:, None, 4 * h + p, :]
                            else:
                                cf = t_uc[h][:, None, ci, :]
                                ci += 1
                            cf = cf.broadcast_to([128, D, BAND])
                            if first:
                                eng.tensor_tensor(
                                    t_samp[:, h, :, :], src, cf, OP.mult)
                                first = False
                                continue
                            eng.tensor_tensor(ptg[:, c, :, :], src, cf, OP.mult)
                        c0 = 1 if gi == 0 else 0
                        if k - c0 <= 0:
                            continue
                        sv = t_samp[:, h, None, :, :].broadcast_to(
                            [128, k - c0, D, BAND])
                        eng.tensor_tensor(sv, sv, ptg[:, c0:k, :, :], OP.add)

            # ================= G: attn transpose + out-proj + residual ======
            with tc.tile_pool(name="outp", bufs=1) as Po, \
                 tc.tile_pool(name="rota", bufs=4) as Pra:
                t_out = Po.tile([128, BAND, COUT], F32)
                for yc in range(BAND):
                    aT = []
                    for k in range(2):
                        pT = PSa.tile([128, 128], BF16, tag="trT", name="pT",
                                      padded_shape=[128, 512])
                        src = t_samp[:, 4 * k:4 * (k + 1), :, yc].rearrange(
                            "x h d -> x (h d)")
                        nc.tensor.transpose(pT[:], src, t_idb[:])
                        sT = Pra.tile([128, 128], BF16, tag="aT", name="sT")
                        nc.vector.tensor_copy(sT[:], pT[:])
                        aT.append(sT)
                    pU = PSb.tile([128, COUT], F32, tag="proj", name="pU")
                    nc.tensor.matmul(pU[:], aT[0][:], t_wo[:, 0, :],
                                     start=True, stop=False)
                    nc.tensor.matmul(pU[:], aT[1][:], t_wo[:, 1, :],
                                     start=False, stop=False)
                    nc.tensor.matmul(pU[:], t_ones[:], t_bo[:],
                                     start=False, stop=True)
                    nc.vector.tensor_tensor(
                        t_out[:, yc, :], pU[:], t_q2[:, yc, :], OP.add)
                nc.sync.dma_start(
                    d_out[:].rearrange("(y x) c -> x y c", x=128), t_out[:])
            _q2_cm.__exit__(None, None, None)

    nc.finalize()
    return nc


def _make_inputs(inputs, meta):
    query = np.ascontiguousarray(inputs["query"], dtype=np.float32)
    value = np.ascontiguousarray(inputs["value"], dtype=np.float32)
    BH, halo_t = meta["BH"], meta["halo_t"]
    cb = np.zeros((128, 64), np.float32)
    cb[:, 0:32] = meta["basex"].reshape(-1)[None, :]
    cb[:, 32:64] = meta["basey"].reshape(-1)[None, :]
    woa = np.concatenate([np.asarray(inputs["W_off"], np.float32),
                          np.asarray(inputs["W_attn"], np.float32)], axis=1)
    boa = np.concatenate([np.asarray(inputs["b_off"], np.float32),
                          np.asarray(inputs["b_attn"], np.float32)])[None, :]
    consts = {
        "wval": np.asarray(inputs["W_val"], np.float32).astype(
            ml_dtypes.bfloat16),
        "woa": np.ascontiguousarray(woa),
        "wout": np.asarray(inputs["W_out"], np.float32).astype(
            ml_dtypes.bfloat16),
        "ident": np.eye(128, dtype=np.float32),
        "identb": np.eye(128, dtype=np.float32).astype(ml_dtypes.bfloat16),
        "cb": cb,
        "onesrow": np.ones((1, 128), np.float32),
        "boa": np.ascontiguousarray(boa),
        "bvrow": np.asarray(inputs["b_val"], np.float32)[None, :],
        "borow": np.asarray(inputs["b_out"], np.float32)[None, :],
        "zgap": np.zeros((16, BH * D), ml_dtypes.bfloat16),
    }
    in_maps = []
    for b in range(query.shape[0]):
        vimg = value[b].reshape(H, W, CIN)
        qimg = query[b].reshape(H, W, CIN)
        for i in range(NB):
            lo = i * BAND - halo_t
            pad = np.zeros((BH, W, CIN), np.float32)
            s0, s1 = max(0, lo), min(H, lo + BH)
            pad[s0 - lo:s1 - lo] = vimg[s0:s1]
            m = dict(consts)
            m["valpad"] = pad.reshape(BH * W, CIN).astype(ml_dtypes.bfloat16)
            m["qband"] = np.ascontiguousarray(
                qimg[i * BAND:(i + 1) * BAND].reshape(BAND * W, CIN))
            in_maps.append(m)
    return in_maps


def _run(inputs, trace=False):
    query = np.ascontiguousarray(inputs["query"], dtype=np.float32)
    h, w = int(inputs["h"]), int(inputs["w"])
    assert (h, w) == (H, W), (h, w)
    bs = query.shape[0]
    assert bs * NB == 8

    meta = _host_meta(query, np.asarray(inputs["W_off"], np.float32),
                      np.asarray(inputs["b_off"], np.float32))
    nc = _build_program(meta)
    in_maps = _make_inputs(inputs, meta)

    res = run_bass_kernel_spmd(nc, in_maps, core_ids=list(range(8)),
                               trace=trace)
    out = np.empty((bs, NQ, COUT), np.float32)
    for b in range(bs):
        for i in range(NB):
            out[b, i * BAND * W:(i + 1) * BAND * W] = \
                res.results[b * NB + i]["out"]
    return out, res


def kernel(**inputs):
    out, _ = _run(inputs, trace=False)
    return out

